# revision 32
# baseline (speedup 1.0000x reference)
"""Trainium2 Bass kernel for nn_ECA_69544110457542.

Math (per row r=(b,t)):
  dyn   = x[:, :31] @ Wd + bd
  value = x[:, 31] * Wv[0] + bv
  xhn   = [dyn | human@Wh+bh | nature@Wn+bn]                      (768 ch)
  pre_j = sum_k cw[t,k] * xhn[perm[ainv[j]+k-3]] + conv_b[t]      (j<256)
  sel   = softmax(relu(pre))
  out   = 0.5*(dyn*sel) @ Wvd1  +  0.5*dyn @ Wvd1 + value @ Wvd2 + bvd
          `------ device ------'  `------- folded into wdf (host) -----'

Sharding is T-MAJOR data parallel: core c takes all 256 batches for the 8
timesteps t in [8c, 8c+8); each 128-row tile holds one fixed t.  That makes
cw[t,k] a per-tile constant, so the conv combine folds into the MATMULS:
DVE pre-scales the activation tile by cw[t,k] (5 copies, shared by the two
batch-half tiles of each t) and PE accumulates all 5 gathered-conv matmuls
into a single [128,256] PSUM `pre`.  The 1280-column PSUM drain + 9-op
combine chain of the row-major variant disappears; conv_b[t] rides the exp
bias, relu runs after exp as max(exp,1) (exact: exp(relu(x))=max(exp(x),1))
whose DVE accum_out is the softmax denominator.

Two-stage pipeline over 16 tiles:
  stage 1 (tile i):   PE: [wdyn|wfold] matmul + 10 accumulating G matmuls.
  stage 2 (tile i-1): ACT exp (reads PSUM, bias=conv_b[t]); DVE max+accum,
                      reciprocal, gate; PE transposes z, zT @ 0.5*Wvd1 into
                      the fold PSUM; ACT drains zT and the finished out
                      tile; Pool issues the output DMA.
DVE also builds the next t's scaled activation copies each even tile, one
t ahead of PE's use.

PSUM (8 banks): pre [P,256]x3, pdf [P,512]x3, ptz x2.  SBUF working tiles
use bufs>=4 so no WAR semaphores.  Bacc.finalize() splits any multi-wait
instruction into EventSemaphore preludes (1-wait ISA limit).
"""

import sys

sys.path.insert(0, "/opt/trn_rl_repo")

from contextlib import ExitStack

import ml_dtypes
import numpy as np

import concourse.bacc as bacc
import concourse.bass as bass
import concourse.tile as tile
from concourse import mybir
from concourse.bass_utils import run_bass_kernel_spmd

# problem constants
B, T, E = 256, 64, 256
XS, DS = 32, 31
HT, NT_ = 80, 80
C = 3 * E
KW = 5
NCORES = 8
TPC = T // NCORES          # 8 timesteps per core
R = B * TPC                # 2048 rows per core (t-major: r = t*256 + b)
P = 128
NTILES = R // P            # 16 (two batch-halves per t)
AK = XS + 1 + HT + NT_     # 193 act rows: x(32) | ones | h(80) | n(80)
K2 = AK - 128              # 65
NG = KW * E                # 1280 gathered columns

# packed-constants layout, fp32 slot offsets in [128, WPACK]
O_WG1 = 0                   # bf16 [128, 1280] (K rows 0:128, k-major cols)
O_WG2 = 640                 # bf16 [65, 1280]  (K rows 128:193)
O_WDF = 1280                # bf16 [128, 512]: wdyn | wfold (rows 33: zero)
O_WV1 = 1536                # bf16 [128, 512]: 0.5*Wvd1 rows 0:128 | 128:256
O_IDB = 1792                # bf16 identity [128, 128]
O_CWB = 1856                # fp32 [128, 8, 6]: per-t cw[t,0..4], conv_b[t]
WPACK = 1904

F32 = mybir.dt.float32
BF16 = mybir.dt.bfloat16
MULT = mybir.AluOpType.mult
ADD = mybir.AluOpType.add
EXP = mybir.ActivationFunctionType.Exp

_NC_CACHE = None
LAST_RESULTS = None
TRACE = False


def _patched_drain_and_barrier(self, tick_clock, wait_clock):
    # The stock kernel-tail drain puts every processor's final-tick wait on a
    # single Drain instruction; this walrus build rejects multi-wait
    # instructions, so spread the waits over a chain of drains instead.
    import bass_rust as _br
    from concourse.vector_clock import ScopedClock

    nc = self.nc
    drain_inst = nc.sync.drain()
    wait_clock.add_sem_waits(
        drain_inst.ins, ScopedClock({None: tick_clock.global_clock})
    )
    si = drain_inst.ins.sync_info
    if si is not None and len(si.on_wait) > 1:
        waits = list(si.on_wait)
        drain_inst.ins.sync_info = _br.SyncInfo(
            on_wait=[waits[0]], on_update=list(si.on_update)
        )
        for w in waits[1:]:
            d2 = nc.sync.drain()
            d2.ins.sync_info = _br.SyncInfo(on_wait=[w], on_update=[])
    nc.all_engine_barrier()
    assert self.sems is not None
    popped = nc._tile_sem_poison_stack.pop()
    assert popped is self._sem_poison
    nc.clear_and_free_semaphores(list(self.sems.allocated().values()))
    nc.all_engine_barrier()


tile.TileContext._drain_and_barrier = _patched_drain_and_barrier


def _build_nc():
    # Bacc (not plain Bass): its finalize() runs the passes that split
    # multi-wait instructions into EventSemaphore preludes.
    nc = bacc.Bacc()
    actb_d = nc.dram_tensor("actb", [P, 2 * R], BF16, kind="ExternalInput")
    wpack_d = nc.dram_tensor("wpack", [P, WPACK], F32, kind="ExternalInput")
    out_d = nc.dram_tensor("out", [R, E], F32, kind="ExternalOutput")
    actb3 = actb_d[:, :].rearrange("p (two r) -> p two r", two=2)

    with tile.TileContext(nc) as tc, ExitStack() as ctx:
        consts = ctx.enter_context(tc.tile_pool(name="consts", bufs=1))
        psP = ctx.enter_context(tc.tile_pool(name="psP", bufs=3, space="PSUM"))
        psD = ctx.enter_context(tc.tile_pool(name="psD", bufs=3, space="PSUM"))
        psT = ctx.enter_context(tc.tile_pool(name="psT", bufs=2, space="PSUM"))
        pak = ctx.enter_context(tc.tile_pool(name="pak", bufs=2))
        pc = ctx.enter_context(tc.tile_pool(name="pc", bufs=4))
        pz = ctx.enter_context(tc.tile_pool(name="pz", bufs=4))
        pzT = ctx.enter_context(tc.tile_pool(name="pzT", bufs=4))
        po = ctx.enter_context(tc.tile_pool(name="po", bufs=4))
        psm = ctx.enter_context(tc.tile_pool(name="psm", bufs=4))

        wp = consts.tile([P, WPACK], F32)
        ab = consts.tile([P, 2, R], BF16)

        # input DMAs split between the idle SP sequencer (HWDGE, ~565ns per
        # issue) and ACT (~667ns, idle until its first stage-2 op); never on
        # the Pool Q7 whose SWDGE generation is ~1us each and serialized
        nc.sync.dma_start(ab[:, :, 0 : R // 4], actb3[:, :, 0 : R // 4])
        nc.sync.dma_start(wp[:, 0:640], wpack_d[:, 0:640])
        nc.sync.dma_start(wp[:, 640:1280], wpack_d[:, 640:1280])
        nc.sync.dma_start(ab[:, :, R // 4 : R // 2], actb3[:, :, R // 4 : R // 2])
        nc.scalar.dma_start(wp[:, O_WDF:WPACK], wpack_d[:, O_WDF:WPACK])
        nc.scalar.dma_start(ab[:, :, R // 2 : 3 * R // 4],
                            actb3[:, :, R // 2 : 3 * R // 4])
        nc.scalar.dma_start(ab[:, :, 3 * R // 4 : R],
                            actb3[:, :, 3 * R // 4 : R])

        wpb = wp[:].bitcast(BF16)
        wg1 = wpb[:, 2 * O_WG1 : 2 * O_WG1 + NG]
        wg2 = wpb[0:K2, 2 * O_WG2 : 2 * O_WG2 + NG]
        wdf = wpb[:, 2 * O_WDF : 2 * O_WDF + 512]
        wv1a = wpb[:, 2 * O_WV1 : 2 * O_WV1 + E]
        wv1b = wpb[:, 2 * O_WV1 + E : 2 * O_WV1 + 2 * E]
        identb = wpb[:, 2 * O_IDB : 2 * O_IDB + P]
        cwb = wp[:, O_CWB:WPACK].rearrange("p (t s) -> p t s", s=6)

        # warmup: ACT/DVE observe the small-weights DMA once (PE's first
        # matmuls carry the weight-DMA waits themselves; Bacc splits any
        # tile-0 multi-waits into EventSemaphores)
        at = psm.tile([P, 1], F32, tag="wm")
        nc.scalar.copy(at[:], cwb[:, 0, 5:6])
        dt_ = psm.tile([P, 1], F32, tag="wm2")
        nc.vector.tensor_copy(dt_[:], cwb[:, 0, 5:6])

        def make_ak(t):
            # cw-scaled copies of t's activation block (both K-splits),
            # shared by the two batch-half tiles of this t.  The host
            # normalizes cw by cw[t,2], so k=2 uses the raw activations and
            # the cw[t,2] factor rides the exp's scale= for free.
            tc_ = slice(t * 2 * P, (t + 1) * 2 * P)
            aks = {}
            for k in (0, 1, 3, 4):
                akt = pak.tile([P, 2, 2 * P], BF16, tag=f"ak{k}", name=f"ak{k}_{t}")
                # one op covers both K-splits (partitions 65:128 of the
                # second split are junk the matmuls never read)
                nc.vector.tensor_scalar_mul(akt[:, :, :], ab[:, :, tc_],
                                            cwb[:, t, k : k + 1])
                aks[k] = akt
            return aks

        aks_by_t = {0: make_ak(0)}
        st1 = {}
        st2 = {}

        for it in range(NTILES + 2):
            # ---- stage 1 PE (tile it): dyn/fold + accumulated G ---------
            if it < NTILES:
                i = it
                t, h = divmod(i, 2)
                hs = slice(h * P, (h + 1) * P)
                rows = slice(i * P, (i + 1) * P)
                b0 = ab[:, 0, rows]
                aks = aks_by_t[t]

                pdf = psD.tile([P, 512], F32, tag="pdf")
                pre = psP.tile([P, 256], F32, tag="pre")

                nc.tensor.matmul(pdf[:, 0:512], b0, wdf,
                                 start=True, stop=True, skip_group_check=True)
                ghs = slice(t * 2 * P + h * P, t * 2 * P + (h + 1) * P)
                for k in range(KW):
                    l0 = ab[:, 0, ghs] if k == 2 else aks[k][:, 0, hs]
                    l1 = ab[0:K2, 1, ghs] if k == 2 else aks[k][0:K2, 1, hs]
                    nc.tensor.matmul(pre[:, 0:256], l0,
                                     wg1[:, k * E : (k + 1) * E],
                                     start=(k == 0), stop=False,
                                     skip_group_check=True)
                    nc.tensor.matmul(pre[:, 0:256], l1,
                                     wg2[:, k * E : (k + 1) * E],
                                     start=False, stop=(k == KW - 1),
                                     skip_group_check=True)
                st1[i] = (t, pre, pdf)

            # ---- stage 2b PE (tile it-2): zT @ 0.5*Wvd1 into pdf --------
            if it >= 2:
                zTs2, pdf2 = st2[it - 2]
                nc.tensor.matmul(pdf2[:, 256:512], zTs2[:, 0, :], wv1a,
                                 start=False, stop=False, skip_group_check=True)
                nc.tensor.matmul(pdf2[:, 256:512], zTs2[:, 1, :], wv1b,
                                 start=False, stop=True, skip_group_check=True)

            # ---- stage 2a (tile it-1): softmax + gate + transpose -------
            if 1 <= it <= NTILES:
                j = it - 1
                tj, prej, pdfj = st1.pop(j)
                # exp reads pre straight from PSUM; conv_b[t] rides the bias.
                # exp(relu(x)) == max(exp(x), 1): the relu runs after exp as
                # a DVE max whose accum_out is the softmax denominator.
                ex0 = pc.tile([P, E], BF16, tag="ex0")
                nc.scalar.activation(ex0[:], prej[:, 0:256], func=EXP,
                                     scale=cwb[:, tj, 2:3],
                                     bias=cwb[:, tj, 5:6])
                exm = pc.tile([P, E], BF16, tag="exm")
                ssum = psm.tile([P, 1], F32, tag="ssum")
                nc.vector.tensor_scalar(exm[:], ex0[:], 1.0, 0.0,
                                        op0=mybir.AluOpType.max,
                                        op1=mybir.AluOpType.add,
                                        accum_out=ssum[:])
                sinv = psm.tile([P, 1], F32, tag="sinv")
                nc.vector.reciprocal(sinv[:], ssum[:])
                z = pz.tile([P, E], BF16, tag="z")
                nc.vector.scalar_tensor_tensor(z[:], exm[:], sinv[:],
                                               pdfj[:, 0:256], op0=MULT, op1=MULT)
                ptz = psT.tile([P, 2, P], BF16, tag="ptz")
                nc.tensor.transpose(ptz[:, 0, :], z[:, 0:128], identb)
                nc.tensor.transpose(ptz[:, 1, :], z[:, 128:256], identb)
                zTs = pzT.tile([P, 2, P], BF16, tag="zTs", name=f"zTs{j}")
                nc.scalar.copy(zTs[:], ptz[:])
                st2[j] = (zTs, pdfj)

            # ---- DVE builds next t's scaled activations, one t ahead ----
            if it < NTILES and it % 2 == 0 and (it // 2) + 1 < TPC:
                aks_by_t[(it // 2) + 1] = make_ak((it // 2) + 1)
                aks_by_t.pop((it // 2) - 1, None)

            # ---- stage 2c (tile it-2): out drain + DMA ------------------
            if it >= 2:
                zTs2, pdf2 = st2.pop(it - 2)
                ob = po.tile([P, E], F32, tag="ob", name=f"ob{it - 2}")
                nc.scalar.copy(ob[:], pdf2[:, 256:512])
                nc.gpsimd.dma_start(
                    out_d[(it - 2) * P : (it - 1) * P, :], ob[:]
                )

    nc.finalize()
    return nc


def _host_prep(x, human, nature, perm, Wv, bv, Wd, bd, Wh, bh, Wn, bn,
               conv_w, conv_b, Wvd, bvd):
    f = np.float32
    bf = ml_dtypes.bfloat16
    x = np.asarray(x, f)
    human = np.asarray(human, f)
    nature = np.asarray(nature, f)
    Wv = np.asarray(Wv, f); bv = np.asarray(bv, f)
    Wd = np.asarray(Wd, f); bd = np.asarray(bd, f)
    Wh = np.asarray(Wh, f); bh = np.asarray(bh, f)
    Wn = np.asarray(Wn, f); bn = np.asarray(bn, f)
    conv_w = np.asarray(conv_w, f)
    conv_b = np.asarray(conv_b, f)
    Wvd = np.asarray(Wvd, f); bvd = np.asarray(bvd, f)
    perm = np.asarray(perm).astype(np.int64)

    Wvd1 = Wvd[:E, :]
    Wvd2 = Wvd[E:, :]

    # t-major activation rows: r = t_global*B + b
    acts = np.concatenate(
        [
            x.reshape(B * T, XS),
            np.ones((B * T, 1), f),
            human.reshape(B * T, HT),
            nature.reshape(B * T, NT_),
        ],
        axis=1,
    )
    acts_tm = np.ascontiguousarray(
        acts.reshape(B, T, AK).transpose(1, 0, 2).reshape(B * T, AK)
    )
    actsT = np.ascontiguousarray(acts_tm.T)  # [193, T*B]
    actb = np.zeros((P, 2, B * T), bf)
    actb[:, 0, :] = actsT[0:128]
    actb[0:K2, 1, :] = actsT[128:AK]

    wpack = np.zeros((P, WPACK), f)
    wpv = wpack.view(bf)  # bf16 alias [128, 2*WPACK]

    # dyn | folded-linear weights (rows 33:128 zero so the matmul can use
    # the full 128-row stationary block)
    wdf = np.zeros((128, 512), f)
    wdf[0:DS, 0:E] = Wd
    wdf[32, 0:E] = bd
    wdf[0:DS, E:512] = 0.5 * (Wd @ Wvd1)
    wdf[31, E:512] = Wv[0] @ Wvd2
    wdf[32, E:512] = 0.5 * (bd @ Wvd1) + bv @ Wvd2 + bvd
    wpv[:, 2 * O_WDF : 2 * O_WDF + 512] = wdf.astype(bf)

    # gathered conv weights (bf16), k-major column blocks
    ainv = np.argsort(perm)
    Wg = np.zeros((AK, NG), f)
    for k in range(KW):
        pos = ainv[:E] + k - 3
        for j in range(E):
            pj = pos[j]
            if 0 <= pj < C:
                c = perm[pj]
                col = k * E + j
                if c < E:
                    Wg[0:DS, col] = Wd[:, c]
                    Wg[32, col] = bd[c]
                elif c < 2 * E:
                    Wg[33:113, col] = Wh[:, c - E]
                    Wg[32, col] = bh[c - E]
                else:
                    Wg[113:193, col] = Wn[:, c - 2 * E]
                    Wg[32, col] = bn[c - 2 * E]
    wpv[:, 2 * O_WG1 : 2 * O_WG1 + NG] = Wg[0:128].astype(bf)
    wpv[0:K2, 2 * O_WG2 : 2 * O_WG2 + NG] = Wg[128:AK].astype(bf)

    # 0.5 * Wvd1 (bf16), split into two K-chunks
    wv1 = (0.5 * Wvd1).astype(bf)
    wpv[:, 2 * O_WV1 : 2 * O_WV1 + E] = wv1[0:128]
    wpv[:, 2 * O_WV1 + E : 2 * O_WV1 + 2 * E] = wv1[128:256]

    wpv[:, 2 * O_IDB : 2 * O_IDB + P] = np.eye(P, dtype=bf)
    return actb, wpack, conv_w[:, 0, :], conv_b


def kernel(**inputs):
    global _NC_CACHE, LAST_RESULTS
    actb, wpack, cwt, cbt = _host_prep(**inputs)

    if _NC_CACHE is None:
        _NC_CACHE = _build_nc()
    nc = _NC_CACHE

    in_maps = []
    for ci in range(NCORES):
        sb = np.ascontiguousarray(actb[:, :, ci * R : (ci + 1) * R]).reshape(
            P, 2 * R
        )
        wpc = wpack.copy()
        cw6 = np.zeros((TPC, 6), np.float32)
        cwc = cwt[ci * TPC : (ci + 1) * TPC]            # [TPC, 5]
        cw6[:, 0:KW] = cwc / cwc[:, 2:3]                 # normalized by cw[t,2]
        cw6[:, 2] = cwc[:, 2]                            # exp scale
        cw6[:, 5] = cbt[ci * TPC : (ci + 1) * TPC]       # exp bias
        wpc[:, O_CWB:WPACK] = np.broadcast_to(cw6.reshape(1, -1), (P, TPC * 6))
        in_maps.append({"actb": sb, "wpack": wpc})

    res = run_bass_kernel_spmd(nc, in_maps, core_ids=list(range(NCORES)), trace=TRACE)
    LAST_RESULTS = res

    # de-shard: core c's rows are (t_local, b) for t_global = c*TPC + t_local
    out = np.empty((B, T, E), np.float32)
    for ci in range(NCORES):
        blk = res.results[ci]["out"].reshape(TPC, B, E)
        out[:, ci * TPC : (ci + 1) * TPC, :] = blk.transpose(1, 0, 2)
    return out
